# revision 1
# baseline (speedup 1.0000x reference)
"""Trainium2 Bass kernel for batched DMV inside algorithm (nn_DMV_79190607004378).

Strategy
--------
Pure data parallelism: 1024 sentences -> 8 NeuronCores x 128 sentences.
On each core, one sentence per SBUF partition; all DP charts are [128, 64*64]
f32 SBUF tensors (one row-major [h, e] chart per partition).

The log-domain inside DP (logsumexp over split points) is run in a *scaled
probability domain*: every chart entry X[h,e] of span width v = |e-h| is
stored as  Xp = exp(X + ALPHA*v).  Because span widths add exactly at every
DP combination, the ALPHA*v scale telescopes: each width step reduces to
strided band multiplies + segmented sum-reduces on the Vector engine --
no per-element exp/log anywhere in the hot loop.  f32 keeps >=7 digits:
validated max rel err ~2e-7 vs the log-domain reference.

Left and right directions are fused into single DVE instructions via
paired-group access patterns (an extra AP dimension of size 2 spanning the
two charts, possible because all 8 charts are co-allocated in one SBUF
tensor at fixed 4096-column offsets chosen to make every pair gap positive).

Tag-pair parameter gathers (trans_param[th[h], th[m]]), exp() of the tiny
parameter tensors, and the final log() are done host-side (they touch only
the small parameter tensors / [B,64] arrays); everything O(B*n^3) runs on
device.
"""
import numpy as np
import bass_rust
import concourse.bass as bass
import concourse.mybir as mybir

F32 = mybir.dt.float32
N = 64
B = 1024
NCORES = 8
BPC = B // NCORES  # 128 sentences per core == SBUF partition count
ALPHA = 5.0

# chart column bases inside the big SBUF tensor (order matters: it makes all
# paired-group AP gaps positive -- see build_nc)
CRG, CLG, IRB, ILB, FLB, FRB, FRT, FLT = (i * 4096 for i in range(8))
CH_F = 8 * 4096

# input-tensor column layout (host-assembled, per core)
TRNS_R = 0          # exp(trans_r[h,m] + ALPHA), row-major h*64+m
TRNS_L = 4096       # exp(trans_l[h,m] + ALPHA), row-major h*64+m  (h = head)
GO = 8192           # exp(goR_hc) | exp(goL_hc)
ST = 8320           # exp(stR_hc) | exp(stL_hc)
DCG = 8448          # exp(goR_nc) | exp(goL_nc)      (CRG/CLG diag init)
DF = 8576           # exp(stL_nc) | exp(stR_nc)      (FL/FR diag init, L first)
DFT = 8704          # exp(stR_nc) | exp(stL_nc)      (FRt/FLt diag init)
ROOTC = 8832        # exp(root[th]) * (r < len)
OHL = 8896          # onehot(e == len-1)
ST2 = 8960          # stR_hc/stR_nc | stL_hc/stL_nc (for FRt'/FLt' charts)
INP_F = 9088

MUL = mybir.AluOpType.mult
ADD = mybir.AluOpType.add


def mk_ap(t, offset, dims):
    """Custom AP on an SBUF tensor t ([128, F]): free (step,count) dims + element offset."""
    a = t[:]
    fsz = a.ap[0][0]
    a.ap = bass_rust.VecI64Pair([[fsz, 128]] + [list(d) for d in dims])
    a.offset = offset
    return a


def build_nc(n_repeats: int = 1):
    """v2 program: Vector engine runs the phase-1 chain, reduces, and chart
    updates; GPSIMD concurrently computes the phase-2 bulk band (splits that
    only involve width<w entries) and the transposed-F chart writes.
    n_repeats>1 re-runs the whole DP (for differential HW timing)."""
    nc = bass.Bass()
    inp = nc.dram_tensor("inp", [BPC, INP_F], F32, kind="ExternalInput")
    outp = nc.dram_tensor("out", [BPC, 1], F32, kind="ExternalOutput")

    charts = nc.alloc_sbuf_tensor("charts", [128, CH_F], F32)
    inps = nc.alloc_sbuf_tensor("inps", [128, INP_F], F32)
    bandA = nc.alloc_sbuf_tensor("bandA", [128, 4096], F32)
    bandB = nc.alloc_sbuf_tensor("bandB", [128, 2048], F32)
    segA = nc.alloc_sbuf_tensor("segA", [128, 128], F32)
    segB = nc.alloc_sbuf_tensor("segB", [128, 128], F32)
    Pt = nc.alloc_sbuf_tensor("Pt", [128, 1], F32)

    with (
        nc.Block() as block,
        nc.semaphore("dsem") as dsem,
        nc.semaphore("vsem") as vsem,
        nc.semaphore("gin") as gin,    # v -> g: charts/segB of width w ready
        nc.semaphore("gout") as gout,  # g -> v: bandB bulk of width w ready
    ):
        @block.sync
        def _(sync):
            sync.dma_start(out=inps[:], in_=inp[:]).then_inc(dsem, 16)
            sync.wait_ge(vsem, 1)
            sync.dma_start(out=outp[:], in_=Pt[:]).then_inc(dsem, 16)

        @block.gpsimd
        def _(g):
            g.wait_ge(dsem, 16)
            for rep in range(n_repeats):
                for w in range(1, N):
                    L = N - w
                    g.wait_ge(gin, rep * N + w)
                    if w >= 2:
                        w1 = w - 1
                        L1 = N - w1
                        g.tensor_tensor(
                            out=mk_ap(charts, FRT + 64 * w1, [(4096 - 63 * w1, 2), (65, L1)]),
                            in0=mk_ap(segB, 0, [(L1, 2), (1, L1)]),
                            in1=mk_ap(inps, ST2, [(64 + w1, 2), (1, L1)]),
                            op=MUL)
                        g.drain()
                        g.tensor_tensor(
                            out=mk_ap(bandB, 0, [(L * w + 1, 2), (w, L), (1, w - 1)]),
                            in0=mk_ap(charts, IRB + 1, [(4096 + 64 * w, 2), (65, L), (1, w - 1)]),
                            in1=mk_ap(charts, FRT + 64 * w + 1, [(4096 - 64 * w, 2), (65, L), (1, w - 1)]),
                            op=MUL)
                        g.drain().then_inc(gout, 1)
                    else:
                        g.drain().then_inc(gout, 1)

        @block.vector
        def _(v):
            v.wait_ge(dsem, 16)
            for rep in range(n_repeats):
                v.memset(mk_ap(charts, FRB, [(1, 4096)]), 0.0)
                v.drain()
                v.tensor_copy(out=mk_ap(charts, CRG, [(4096, 2), (65, 64)]),
                              in_=mk_ap(inps, DCG, [(64, 2), (1, 64)]))
                v.tensor_copy(out=mk_ap(charts, FLB, [(4096, 2), (65, 64)]),
                              in_=mk_ap(inps, DF, [(64, 2), (1, 64)]))
                v.drain().then_inc(gin, 1)

                for w in range(1, N):
                    L = N - w
                    v.tensor_tensor(
                        out=mk_ap(bandA, 0, [(L * w, 2), (w, L), (1, w)]),
                        in0=mk_ap(charts, CRG, [(4096 + 64 * w + 1, 2), (65, L), (1, w)]),
                        in1=mk_ap(charts, FLB + 64 * w + 1, [(4096 - 64 * w - 1, 2), (65, L), (1, w)]),
                        op=MUL)
                    v.drain()
                    v.tensor_reduce(
                        out=mk_ap(segA, 0, [(L, 2), (1, L)]),
                        in_=mk_ap(bandA, 0, [(L * w, 2), (w, L), (1, w)]),
                        axis=mybir.AxisListType.X, op=ADD)
                    v.drain()
                    v.tensor_tensor(
                        out=mk_ap(charts, IRB + w, [(4096 + 63 * w, 2), (65, L)]),
                        in0=mk_ap(segA, 0, [(L, 2), (1, L)]),
                        in1=mk_ap(inps, TRNS_R + w, [(4096 + 63 * w, 2), (65, L)]),
                        op=MUL)
                    # fresh band-B columns: R col w-1 / L col 0 = segA * trans3
                    v.tensor_tensor(
                        out=mk_ap(bandB, w - 1, [(L * w + 1 - w, 2), (w, L)]),
                        in0=mk_ap(segA, 0, [(L, 2), (1, L)]),
                        in1=mk_ap(inps, TRNS_R + w, [(4096 + 63 * w, 2), (65, L)]),
                        op=MUL)
                    v.drain()
                    v.wait_ge(gout, rep * (N - 1) + w)
                    v.tensor_reduce(
                        out=mk_ap(segB, 0, [(L, 2), (1, L)]),
                        in_=mk_ap(bandB, 0, [(L * w, 2), (w, L), (1, w)]),
                        axis=mybir.AxisListType.X, op=ADD)
                    v.drain()
                    v.tensor_tensor(   # CRG/CLG
                        out=mk_ap(charts, CRG + w, [(4096 + 63 * w, 2), (65, L)]),
                        in0=mk_ap(segB, 0, [(L, 2), (1, L)]),
                        in1=mk_ap(inps, GO, [(64 + w, 2), (1, L)]),
                        op=MUL)
                    v.tensor_tensor(   # FR,FL merged via negative pair step
                        out=mk_ap(charts, FRB + w, [(FLB + 64 * w - FRB - w, 2), (65, L)]),
                        in0=mk_ap(segB, 0, [(L, 2), (1, L)]),
                        in1=mk_ap(inps, ST, [(64 + w, 2), (1, L)]),
                        op=MUL)
                    v.drain().then_inc(gin, 1)

            # final: P = sum_r root*(r<len) * FL[r,0] * FR[r,len-1]
            v.tensor_tensor(
                out=mk_ap(bandA, 0, [(64, 64), (1, 64)]),
                in0=mk_ap(charts, FRB, [(64, 64), (1, 64)]),
                in1=mk_ap(inps, OHL, [(0, 64), (1, 64)]),
                op=MUL)
            v.drain()
            v.tensor_reduce(
                out=mk_ap(segA, 0, [(1, 64)]),
                in_=mk_ap(bandA, 0, [(64, 64), (1, 64)]),
                axis=mybir.AxisListType.X, op=ADD)
            v.drain()
            v.tensor_tensor(
                out=mk_ap(segB, 0, [(1, 64)]),
                in0=mk_ap(segA, 0, [(1, 64)]),
                in1=mk_ap(charts, FLB, [(64, 64)]),
                op=MUL)
            v.drain()
            v.tensor_tensor(
                out=mk_ap(segB, 64, [(1, 64)]),
                in0=mk_ap(segB, 0, [(1, 64)]),
                in1=mk_ap(inps, ROOTC, [(1, 64)]),
                op=MUL)
            v.drain()
            v.tensor_reduce(
                out=Pt[:],
                in_=mk_ap(segB, 64, [(1, 64)]),
                axis=mybir.AxisListType.X, op=ADD)
            v.drain().then_inc(vsem, 1)

    nc.finalize()
    return nc


def build_nc_v1(n_repeats: int = 1):
    """v1 program: everything on the Vector engine (no cross-engine sync)."""
    nc = bass.Bass()
    inp = nc.dram_tensor("inp", [BPC, INP_F], F32, kind="ExternalInput")
    outp = nc.dram_tensor("out", [BPC, 1], F32, kind="ExternalOutput")
    charts = nc.alloc_sbuf_tensor("charts", [128, CH_F], F32)
    inps = nc.alloc_sbuf_tensor("inps", [128, INP_F], F32)
    bandA = nc.alloc_sbuf_tensor("bandA", [128, 4096], F32)
    bandB = nc.alloc_sbuf_tensor("bandB", [128, 2048], F32)
    segA = nc.alloc_sbuf_tensor("segA", [128, 128], F32)
    segB = nc.alloc_sbuf_tensor("segB", [128, 128], F32)
    Pt = nc.alloc_sbuf_tensor("Pt", [128, 1], F32)

    with (nc.Block() as block, nc.semaphore("dsem") as dsem, nc.semaphore("vsem") as vsem):
        @block.sync
        def _(sync):
            sync.dma_start(out=inps[:], in_=inp[:]).then_inc(dsem, 16)
            sync.wait_ge(vsem, 1)
            sync.dma_start(out=outp[:], in_=Pt[:]).then_inc(dsem, 16)

        @block.vector
        def _(v):
            v.wait_ge(dsem, 16)
            for rep in range(n_repeats):
                v.memset(mk_ap(charts, FRB, [(1, 4096)]), 0.0)
                v.drain()
                v.tensor_copy(out=mk_ap(charts, CRG, [(4096, 2), (65, 64)]),
                              in_=mk_ap(inps, DCG, [(64, 2), (1, 64)]))
                v.tensor_copy(out=mk_ap(charts, FLB, [(4096, 2), (65, 64)]),
                              in_=mk_ap(inps, DF, [(64, 2), (1, 64)]))
                v.tensor_copy(out=mk_ap(charts, FRT, [(4096, 2), (65, 64)]),
                              in_=mk_ap(inps, DFT, [(64, 2), (1, 64)]))
                v.drain()
                for w in range(1, N):
                    L = N - w
                    v.tensor_tensor(
                        out=mk_ap(bandA, 0, [(L * w, 2), (w, L), (1, w)]),
                        in0=mk_ap(charts, CRG, [(4096 + 64 * w + 1, 2), (65, L), (1, w)]),
                        in1=mk_ap(charts, FLB + 64 * w + 1, [(4096 - 64 * w - 1, 2), (65, L), (1, w)]),
                        op=MUL)
                    v.drain()
                    v.tensor_reduce(
                        out=mk_ap(segA, 0, [(L, 2), (1, L)]),
                        in_=mk_ap(bandA, 0, [(L * w, 2), (w, L), (1, w)]),
                        axis=mybir.AxisListType.X, op=ADD)
                    v.drain()
                    v.tensor_tensor(
                        out=mk_ap(charts, IRB + w, [(4096 + 63 * w, 2), (65, L)]),
                        in0=mk_ap(segA, 0, [(L, 2), (1, L)]),
                        in1=mk_ap(inps, TRNS_R + w, [(4096 + 63 * w, 2), (65, L)]),
                        op=MUL)
                    v.drain()
                    v.tensor_tensor(
                        out=mk_ap(bandB, 0, [(L * w, 2), (w, L), (1, w)]),
                        in0=mk_ap(charts, IRB + 1, [(4096 + 64 * w - 1, 2), (65, L), (1, w)]),
                        in1=mk_ap(charts, FRT + 64 * w + 1, [(4096 - 64 * w - 1, 2), (65, L), (1, w)]),
                        op=MUL)
                    v.drain()
                    v.tensor_reduce(
                        out=mk_ap(segB, 0, [(L, 2), (1, L)]),
                        in_=mk_ap(bandB, 0, [(L * w, 2), (w, L), (1, w)]),
                        axis=mybir.AxisListType.X, op=ADD)
                    v.drain()
                    v.tensor_tensor(
                        out=mk_ap(charts, CRG + w, [(4096 + 63 * w, 2), (65, L)]),
                        in0=mk_ap(segB, 0, [(L, 2), (1, L)]),
                        in1=mk_ap(inps, GO, [(64 + w, 2), (1, L)]), op=MUL)
                    v.tensor_tensor(
                        out=mk_ap(charts, FRB + w, [(FLB + 64 * w - FRB - w, 2), (65, L)]),
                        in0=mk_ap(segB, 0, [(L, 2), (1, L)]),
                        in1=mk_ap(inps, ST, [(64 + w, 2), (1, L)]), op=MUL)
                    v.tensor_tensor(
                        out=mk_ap(charts, FRT + 64 * w, [(4096 - 63 * w, 2), (65, L)]),
                        in0=mk_ap(segB, 0, [(L, 2), (1, L)]),
                        in1=mk_ap(inps, ST, [(64 + w, 2), (1, L)]), op=MUL)
                    v.drain()
            v.tensor_tensor(out=mk_ap(bandA, 0, [(64, 64), (1, 64)]),
                            in0=mk_ap(charts, FRB, [(64, 64), (1, 64)]),
                            in1=mk_ap(inps, OHL, [(0, 64), (1, 64)]), op=MUL)
            v.drain()
            v.tensor_reduce(out=mk_ap(segA, 0, [(1, 64)]),
                            in_=mk_ap(bandA, 0, [(64, 64), (1, 64)]),
                            axis=mybir.AxisListType.X, op=ADD)
            v.drain()
            v.tensor_tensor(out=mk_ap(segB, 0, [(1, 64)]), in0=mk_ap(segA, 0, [(1, 64)]),
                            in1=mk_ap(charts, FLB, [(64, 64)]), op=MUL)
            v.drain()
            v.tensor_tensor(out=mk_ap(segB, 64, [(1, 64)]), in0=mk_ap(segB, 0, [(1, 64)]),
                            in1=mk_ap(inps, ROOTC, [(1, 64)]), op=MUL)
            v.drain()
            v.tensor_reduce(out=Pt[:], in_=mk_ap(segB, 64, [(1, 64)]),
                            axis=mybir.AxisListType.X, op=ADD)
            v.drain().then_inc(vsem, 1)
    nc.finalize()
    return nc


def prep_core_inputs(tag_array, len_array, root_param, trans_param, dec_param):
    """Host preprocessing: tag gathers + exp + per-core input assembly."""
    NC_, HC_, GO_, STOP_, LEFT_, RIGHT_ = 0, 1, 0, 1, 0, 1
    th = np.asarray(tag_array)
    ln = np.asarray(len_array)
    tp = np.asarray(trans_param, np.float32)[..., 0]          # [T,T,2]
    dec = np.asarray(dec_param, np.float32)                   # [T,2,2,2]
    root = np.asarray(root_param, np.float32)

    trans_rp = np.exp(tp[th[:, :, None], th[:, None, :], RIGHT_] + ALPHA, dtype=np.float32)
    trans_lp = np.exp(tp[th[:, :, None], th[:, None, :], LEFT_] + ALPHA, dtype=np.float32)
    d = np.exp(dec[th], dtype=np.float32)                     # [B,N,2,2,2]
    goR_nc, goR_hc = d[:, :, RIGHT_, NC_, GO_], d[:, :, RIGHT_, HC_, GO_]
    goL_nc, goL_hc = d[:, :, LEFT_, NC_, GO_], d[:, :, LEFT_, HC_, GO_]
    stR_nc, stR_hc = d[:, :, RIGHT_, NC_, STOP_], d[:, :, RIGHT_, HC_, STOP_]
    stL_nc, stL_hc = d[:, :, LEFT_, NC_, STOP_], d[:, :, LEFT_, HC_, STOP_]
    ar = np.arange(N)
    rootp = np.exp(root[th], dtype=np.float32) * (ar[None, :] < ln[:, None])
    onehot = (ar[None, :] == (ln - 1)[:, None]).astype(np.float32)

    # fold st_nc[child] into the trans tables: the I' charts then store
    # I*st_nc[child-end], making the fresh band-B columns computable from the
    # split-sum directly (one less pipeline fence per width); the transposed-F
    # charts compensate via st_hc/st_nc ratios (ST2).
    trans3_r = trans_rp * stR_nc[:, None, :]
    trans3_l = trans_lp * stL_nc[:, None, :]
    inp = np.empty((B, INP_F), np.float32)
    inp[:, TRNS_R:TRNS_R + 4096] = trans3_r.reshape(B, 4096)
    inp[:, TRNS_L:TRNS_L + 4096] = trans3_l.reshape(B, 4096)
    inp[:, ST2:ST2 + 64] = stR_hc / stR_nc
    inp[:, ST2 + 64:ST2 + 128] = stL_hc / stL_nc
    inp[:, GO:GO + 64] = goR_hc
    inp[:, GO + 64:GO + 128] = goL_hc
    inp[:, ST:ST + 64] = stR_hc
    inp[:, ST + 64:ST + 128] = stL_hc
    inp[:, DCG:DCG + 64] = goR_nc
    inp[:, DCG + 64:DCG + 128] = goL_nc
    inp[:, DF:DF + 64] = stL_nc
    inp[:, DF + 64:DF + 128] = stR_nc
    inp[:, DFT:DFT + 64] = stR_nc
    inp[:, DFT + 64:DFT + 128] = stL_nc
    inp[:, ROOTC:ROOTC + 64] = rootp
    inp[:, OHL:OHL + 64] = onehot
    return [inp[c * BPC:(c + 1) * BPC] for c in range(NCORES)]


_NC_CACHE = None


def kernel(id_array, tag_array, len_array, root_param, trans_param, dec_param):
    global _NC_CACHE
    if _NC_CACHE is None:
        _NC_CACHE = build_nc()
    nc = _NC_CACHE

    core_inps = prep_core_inputs(tag_array, len_array, root_param, trans_param, dec_param)

    from concourse.bass_utils import run_bass_kernel_spmd
    res = run_bass_kernel_spmd(
        nc, [{"inp": core_inps[c]} for c in range(NCORES)], list(range(NCORES)))
    P = np.concatenate([np.asarray(res.results[c]["out"])[:, 0] for c in range(NCORES)])
    ln = np.asarray(len_array)
    ll = np.log(P) - ALPHA * (ln - 1)
    return ll.astype(np.float32)

